# revision 86
# baseline (speedup 1.0000x reference)
"""Trainium2 Bass kernel for a 2-layer GAT (nn_GAT_70909910057105).

Strategy (8 NeuronCores, SPMD):
  - Core k owns target nodes [128k, 128k+128). Edges bucketed by trg//128 on
    the host (layout-only preprocessing).
  - Edge features ef[src_e, trg_e] are host-gathered, transposed and staged
    bf16 as efT [128, 2, E_pad]; pe = efT.T @ wesum via direct matmuls.
  - Every core builds the FULL node table (all 1024 nodes) from replicated x
    with bf16 matmuls -> no layer-1 collective. Table rows in DRAM:
    [h bf16 x1024 (b-major) | a_src bf16 x16 (b,h) | pad] = 1152 bf16.
  - Per-edge source rows fetched with dma_gather (2304B rows), all gathers
    issued before the compute loop so Q7 descriptor-gen runs ahead.
  - Scores: PSUM-accumulated matmuls (maskT@a_tgt + I@pe + I@a_src); exp via
    leaky trick max(exp(s), exp(0.2 s)) on the Scalar engine, written as
    duplicated bf16 pairs so the DVE message-scaling multiply runs in 2x mode.
  - segment_sum via one-hot mask matmuls into PSUM (as before).
  - Layer 2: x1^T AllGather (bf16) -> full table2 build locally.
"""
import sys

for _p in ("/opt/trn_rl_repo", "/root/.axon_site/_ro/trn_rl_repo"):
    if _p not in sys.path:
        sys.path.insert(0, _p)

import numpy as np
import ml_dtypes
import concourse.bass as bass
import concourse.bacc as bacc
import concourse.tile as tile
from concourse import mybir
from concourse.bass_utils import run_bass_kernel_spmd
from concourse.masks import make_identity

F32 = mybir.dt.float32
BF16 = mybir.dt.bfloat16
I16 = mybir.dt.int16
NPBF = ml_dtypes.bfloat16

N, B, C, H, D = 1024, 4, 256, 4, 64
E = 32768
NC = 8
TPC = N // NC           # target nodes per core = 128
ROW = 1152              # bf16: 1024 h (b-major) | 16 a_src (b,h) | 112 pad
AS_OFF = 1024


# --------------------------------------------------------------------------
# host-side preprocessing (layout / gather only, no arithmetic)
# --------------------------------------------------------------------------

def _pack_idx(vals: np.ndarray) -> np.ndarray:
    n = vals.shape[0]
    assert n % 16 == 0
    blk = vals.astype(np.int16).reshape(n // 16, 16).T
    return np.ascontiguousarray(np.tile(blk, (8, 1)))


def _sb3(w):
    # [R, inner] f32/bf16 -> [128, R//128, inner] with partition = r % 128
    r, inner = w.shape
    return np.ascontiguousarray(
        w.reshape(r // 128, 128, inner).transpose(1, 0, 2)).astype(NPBF)


def _prep(x, edge_features, src_idx, trg_idx,
          Wn1, We1, a_src1, a_tgt1, a_edge1,
          Wn2, We2, a_src2, a_tgt2, a_edge2):
    src = np.asarray(src_idx).astype(np.int64)
    trg = np.asarray(trg_idx).astype(np.int64)
    x = np.asarray(x, dtype=np.float32)
    ef = np.asarray(edge_features, dtype=np.float32)

    buckets = [np.nonzero((trg // TPC) == k)[0] for k in range(NC)]
    # split each bucket by src half so layer-1 gathers can start after only
    # half of the node table is built (separate DRAM tiles per half)
    NQ = 2
    QN = N // NQ
    parts = []
    for k in range(NC):
        eids = buckets[k]
        parts.append([eids[(src[eids] // QN) == q] for q in range(NQ)])
    NC_P = [(max(len(p[q]) for p in parts) + 127) // 128 for q in range(NQ)]
    NC_E = sum(NC_P)
    E_pad = NC_E * 128
    C0_P = [sum(NC_P[:q]) for q in range(NQ)]

    def ablk(a_s, a_t):
        m = np.zeros((C, 8), np.float32)
        for h in range(H):
            m[h * D:(h + 1) * D, h] = np.asarray(a_s)[h]
            m[h * D:(h + 1) * D, 4 + h] = np.asarray(a_t)[h]
        return _sb3(m)



    # x transposed, b-major cols: xT[c%128, c//128, b, n] = x[n, b, c]
    xb = np.ascontiguousarray(x.transpose(2, 1, 0))          # [C, B, N]
    xT = np.ascontiguousarray(
        xb.reshape(2, 128, B, N).transpose(1, 0, 2, 3)).astype(NPBF)
    # group-major copy: xTg[g] = xT[:, :, :, 128g:128(g+1)] contiguous, so
    # each table-build group's load is a single contiguous DMA
    xTg = np.ascontiguousarray(
        xT.reshape(128, 2, B, 8, 128).transpose(3, 0, 1, 2, 4))

    def _sb3f(w):
        r, inner = w.shape
        return np.ascontiguousarray(
            np.asarray(w, np.float32).reshape(r // 128, 128, inner)
            .transpose(1, 0, 2))

    common = {
        "wa1": _sb3(np.asarray(Wn1, np.float32).T),
        "wa2": _sb3(np.asarray(Wn2, np.float32).T),
        "wn1hd": _sb3(np.asarray(Wn1, np.float32)),
        "wn2hd": _sb3(np.asarray(Wn2, np.float32)),
        "ablk1": ablk(a_src1, a_tgt1),
        "ablk2": ablk(a_src2, a_tgt2),
        "we1hd": _sb3f(np.asarray(We1, np.float32)),
        "we2hd": _sb3f(np.asarray(We2, np.float32)),
        "hselr1": _sb3f(np.zeros((C, 16), np.float32) + 0.0),
        "hselr2": _sb3f(np.zeros((C, 16), np.float32) + 0.0),
        "xT": xTg,
    }
    hs1 = np.zeros((C, 16), np.float32)
    hs2 = np.zeros((C, 16), np.float32)
    for b in range(B):
        for h in range(H):
            hs1[h * D:(h + 1) * D, b * H + h] = np.float32(
                np.asarray(a_edge1)[h])
            hs2[h * D:(h + 1) * D, b * H + h] = np.float32(
                np.asarray(a_edge2)[h])
    common["hselr1"] = _sb3f(hs1)
    common["hselr2"] = _sb3f(hs2)

    in_maps = []
    for k in range(NC):
        eo = np.concatenate(parts[k])
        slots = np.concatenate(
            [C0_P[q] * 128 + np.arange(len(parts[k][q])) for q in range(NQ)])
        src_s = np.zeros(E_pad, np.int64)
        src_s[slots] = src[eo] % QN
        src_f = np.zeros(E_pad, np.int64)
        src_f[slots] = src[eo]
        tloc = trg[eo] - k * TPC
        mask = np.zeros((128, E_pad), np.float32)
        maskT = np.zeros((128, E_pad), np.float32)
        mask[slots % 128, (slots // 128) * 128 + tloc] = 1.0
        maskT[tloc, (slots // 128) * 128 + slots % 128] = 1.0
        efg = np.zeros((256, E_pad), np.float32)
        efg[:, slots] = ef[src[eo], trg[eo]].T
        xTloc = np.ascontiguousarray(xT[:, :, :, k * TPC:(k + 1) * TPC])
        m = dict(common)
        m.update({
            "efT": np.ascontiguousarray(
                efg.reshape(2, 128, E_pad).transpose(1, 0, 2)),
            "xTloc": xTloc,
            "isrc": _pack_idx(src_s),
            "isrcf": _pack_idx(src_f),
            "mask": mask.astype(NPBF),
            "maskT": maskT.astype(NPBF),
        })
        in_maps.append(m)
    return in_maps, tuple(NC_P)


# --------------------------------------------------------------------------
# device program
# --------------------------------------------------------------------------

def _build(NC_P: tuple, debug: bool = False):
    NQ = len(NC_P)
    NC_E = sum(NC_P)
    E_pad = NC_E * 128
    # supers grouped per src-quarter: (chunk0, nchunks, quarter)
    supers = []
    for q in range(NQ):
        c0q = sum(NC_P[:q])
        for s in range((NC_P[q] + 7) // 8):
            supers.append((c0q + 8 * s, min(8, NC_P[q] - 8 * s), q))
    n_super = len(supers)
    nc = bacc.Bacc("TRN2", target_bir_lowering=False, debug=False,
                   num_devices=NC)

    efT_in = nc.dram_tensor("efT", [128, 2, E_pad], F32,
                            kind="ExternalInput")
    xT_in = nc.dram_tensor("xT", [8, 128, 2, B, 128], BF16,
                           kind="ExternalInput")
    xTloc_in = nc.dram_tensor("xTloc", [128, 2, B, TPC], BF16,
                              kind="ExternalInput")
    isrc_in = nc.dram_tensor("isrc", [128, E_pad // 16], I16,
                             kind="ExternalInput")
    isrcf_in = nc.dram_tensor("isrcf", [128, E_pad // 16], I16,
                              kind="ExternalInput")
    mask_in = nc.dram_tensor("mask", [128, E_pad], BF16, kind="ExternalInput")
    maskT_in = nc.dram_tensor("maskT", [128, E_pad], BF16,
                              kind="ExternalInput")
    w_in = {
        nm: nc.dram_tensor(nm, [128, 2, inner], BF16, kind="ExternalInput")
        for nm, inner in [
            ("wa1", 256), ("wa2", 256), ("wn1hd", C), ("wn2hd", C),
            ("ablk1", 8), ("ablk2", 8),
        ]
    }
    wf_in = {
        nm: nc.dram_tensor(nm, [128, 2, inner], F32, kind="ExternalInput")
        for nm, inner in [
            ("we1hd", C), ("we2hd", C), ("hselr1", 16), ("hselr2", 16),
        ]
    }
    y_out = nc.dram_tensor("y", [128, B * C], F32, kind="ExternalOutput")
    dbg = {}
    if debug:
        for nm, shape, dt in [("dbg_x1", [128, B * C], BF16),
                              ("dbg_pe", [128, NC_E, 32], F32),
                              ("dbg_tbl", [N, ROW], BF16)]:
            dbg[nm] = nc.dram_tensor(nm, shape, dt, kind="ExternalOutput")

    from contextlib import ExitStack
    with tile.TileContext(nc) as tc:
        with ExitStack() as ctx:
            const = ctx.enter_context(tc.tile_pool(name="const", bufs=1))
            sb = ctx.enter_context(tc.tile_pool(name="sb", bufs=1))
            small = ctx.enter_context(tc.tile_pool(name="small", bufs=3))
            shpool = ctx.enter_context(tc.tile_pool(name="shpool", bufs=3))
            xpool = ctx.enter_context(tc.tile_pool(name="xpool", bufs=4))
            epool = ctx.enter_context(tc.tile_pool(name="epool", bufs=2))
            gpool = ctx.enter_context(
                tc.tile_pool(name="gpool", bufs=n_super))
            erep = ctx.enter_context(tc.tile_pool(name="erep", bufs=3))
            ps_small = ctx.enter_context(
                tc.tile_pool(name="ps_small", bufs=2, space="PSUM"))
            ps_pat = ctx.enter_context(
                tc.tile_pool(name="ps_pat", bufs=2, space="PSUM"))
            ps_t = ctx.enter_context(
                tc.tile_pool(name="ps_t", bufs=1, space="PSUM"))
            ps_out = ctx.enter_context(
                tc.tile_pool(name="ps_out", bufs=1, space="PSUM"))
            ps_den = ctx.enter_context(
                tc.tile_pool(name="ps_den", bufs=1, space="PSUM"))
            dram = ctx.enter_context(
                tc.tile_pool(name="dram", bufs=1, space="DRAM"))

            ident = const.tile([128, 128], BF16)
            make_identity(nc, ident[:])

            # xT first, split per node-group: table-1 build group g starts
            # as soon as its own slice lands
            xg_sb = []
            for g in range(8):
                t = xpool.tile([128, 2, B, 128], BF16, tag="xg")
                nc.sync.dma_start(out=t[:], in_=xT_in[g])
                xg_sb.append(t)
            w_sb = {}
            for nm, t in w_in.items():
                if nm in ("wa1", "wa2"):
                    continue
                inner = t.shape[2]
                w_sb[nm] = const.tile([128, 2, inner], BF16, name=f"w_{nm}",
                                      tag=f"w_{nm}")
                nc.sync.dma_start(out=w_sb[nm][:], in_=t[:])
            for nm, t in wf_in.items():
                inner = t.shape[2]
                w_sb[nm] = const.tile([128, 2, inner], F32, name=f"w_{nm}",
                                      tag=f"w_{nm}")
                nc.sync.dma_start(out=w_sb[nm][:], in_=t[:])
            # bulk staging on the Activation HWDGE ring so the latency-
            # critical table writes (sync ring) don't queue behind it
            xTloc_sb = const.tile([128, 2, B, TPC], BF16)
            nc.sync.dma_start(out=xTloc_sb[:], in_=xTloc_in[:])
            isrc_t = const.tile([128, E_pad // 16], I16)
            nc.sync.dma_start(out=isrc_t[:], in_=isrc_in[:])
            isrcf_t = const.tile([128, E_pad // 16], I16, name="isrcf")
            nc.sync.dma_start(out=isrcf_t[:], in_=isrcf_in[:])
            mask_sb = const.tile([128, E_pad], BF16)
            nc.scalar.dma_start(out=mask_sb[:], in_=mask_in[:])
            maskT_sb = const.tile([128, E_pad], BF16)
            nc.scalar.dma_start(out=maskT_sb[:], in_=maskT_in[:])


            # ---- wesum[c_in, (l, b, h)] f32
            wesum_sb = const.tile([128, 2, 32], F32, name="wes")
            for ct in range(2):
                pw = ps_small.tile([128, 32], F32, space="PSUM", tag="ps",
                                   name="pw")
                for lj, (wehd, hs) in enumerate(
                        [("we1hd", "hselr1"), ("we2hd", "hselr2")]):
                    for kh in range(2):
                        nc.tensor.matmul(
                            out=pw[:, lj * 16:(lj + 1) * 16],
                            lhsT=w_sb[wehd][:, kh, ct * 128:(ct + 1) * 128],
                            rhs=w_sb[hs][:, kh, :],
                            start=(kh == 0), stop=(kh == 1))
                nc.scalar.copy(out=wesum_sb[:, ct, :], in_=pw[:])

            # ---- wa{l} = [Wn.T cols | projected a-cols (device-computed)]
            wa_sb = {}
            for l, (wanm, wnhd, ab) in enumerate(
                    [("wa1", "wn1hd", "ablk1"), ("wa2", "wn2hd", "ablk2")]):
                wt = const.tile([128, 2, 264], BF16, name=f"wt{l}",
                                tag=f"wt{l}")
                nc.sync.dma_start(out=wt[:, :, 0:256], in_=w_in[wanm][:])
                for ct in range(2):
                    pac = ps_small.tile([128, 8], F32, space="PSUM",
                                        tag="ps", name="pac")
                    for kh in range(2):
                        nc.tensor.matmul(
                            out=pac[:],
                            lhsT=w_sb[wnhd][:, kh, ct * 128:(ct + 1) * 128],
                            rhs=w_sb[ab][:, kh, :],
                            start=(kh == 0), stop=(kh == 1))
                    nc.scalar.copy(out=wt[:, ct, 256:264], in_=pac[:])
                wa_sb[wanm] = wt

            # ---- full-table build: rows [h (b,256) | a_src (b,h)]
            # node group g lands in quarter-table g // 2, rows (g % 2) * 128
            def build_table(lhsT_fn, wname, tables):
                gpq = 8 // NQ
                for g in range(8):
                    table = tables[g // gpq]
                    sh = shpool.tile([128, B, 264], BF16, tag="sh")
                    for b in range(B):
                        pool = ps_small if b % 2 == 0 else ps_pat
                        ph = pool.tile([128, 264], F32, space="PSUM",
                                       tag="ps" if b % 2 == 0 else "pat",
                                       name="ph")
                        for ch in range(2):
                            nc.tensor.matmul(
                                out=ph[:], lhsT=lhsT_fn(ch, b, g),
                                rhs=wa_sb[wname][:, ch, :],
                                start=(ch == 0), stop=(ch == 1))
                        if b % 2 == 0:
                            nc.scalar.copy(out=sh[:, b, :], in_=ph[:])
                        else:
                            nc.vector.tensor_copy(out=sh[:, b, :], in_=ph[:])
                    rows = slice((g % gpq) * 128, (g % gpq + 1) * 128)
                    nc.sync.dma_start(
                        out=table[rows, 0:B * C].rearrange(
                            "n (b o) -> n b o", b=B),
                        in_=sh[:, :, 0:256])
                    nc.sync.dma_start(
                        out=table[rows, AS_OFF:AS_OFF + B * H].rearrange(
                            "n (b h) -> n b h", b=B),
                        in_=sh[:, :, 256:260])

            # ---- local a_tgt [t, (b, h)] bf16
            def at_local(loc_fn, wname, tag):
                at = sb.tile([128, B * H], BF16, tag=f"at{tag}",
                             name=f"at{tag}")
                for b in range(B):
                    pa = ps_small.tile([128, 8], F32, space="PSUM", tag="ps",
                                       name="pa")
                    for ch in range(2):
                        nc.tensor.matmul(
                            out=pa[:], lhsT=loc_fn(ch, b),
                            rhs=wa_sb[wname][:, ch, 256:264],
                            start=(ch == 0), stop=(ch == 1))
                    nc.scalar.copy(out=at[:, b * H:(b + 1) * H],
                                   in_=pa[:, 4:8])
                return at

            # ---- edge gathers (layer 1: per src-half table; layer 2: full)
            def issue_gathers(tables, idx_t):
                Gs = []
                for c0, r, q in supers:
                    G = gpool.tile([128, 8, ROW], BF16, tag="G")
                    nc.gpsimd.dma_gather(
                        out_ap=G[:, 0:r, :],
                        in_ap=tables[q % len(tables)][:],
                        idxs_ap=idx_t[:, c0 * 8:(c0 + r) * 8],
                        num_idxs=128 * r, num_idxs_reg=128 * r,
                        elem_size=ROW, single_packet=False)
                    Gs.append(G)
                return Gs

            tbls1 = [dram.tile([N // NQ, ROW], BF16, tag=f"t1{q}",
                               name=f"t1{q}") for q in range(NQ)]

            build_table(
                lambda ch, b, g: xg_sb[g][:, ch, b, :],
                "wa1", tbls1)
            at1 = at_local(lambda ch, b: xTloc_sb[:, ch, b, :], "wa1", 1)
            Gs1 = issue_gathers(tbls1, isrc_t)

            # ---- pe[e, (l, b, h)] f32 for all edge slots (efT streamed)
            pe_sb = sb.tile([128, NC_E, 32], F32)
            for c0, r, _q in supers:
                et = epool.tile([128, 2, 1024], F32, tag="et")
                nc.scalar.dma_start(
                    out=et[:, :, 0:r * 128],
                    in_=efT_in[:, :, c0 * 128:(c0 + r) * 128])
                for j in range(r):
                    pp = ps_small.tile([128, 32], F32, space="PSUM",
                                       tag="ps", name="pp")
                    for ch in range(2):
                        nc.tensor.matmul(
                            out=pp[:],
                            lhsT=et[:, ch, j * 128:(j + 1) * 128],
                            rhs=wesum_sb[:, ch, :],
                            start=(ch == 0), stop=(ch == 1))
                    nc.scalar.copy(out=pe_sb[:, c0 + j, :], in_=pp[:])
            if debug:
                nc.sync.dma_start(out=dbg["dbg_pe"][:], in_=pe_sb[:])

            # ---- edge loop for one layer; s_pre (optional) carries the
            # precomputed maskT@a_tgt + pe base, else computed per super
            def edge_loop(Gs, at, layer, out_dt, s_pre=None):
                out_p = ps_out.tile([128, B * C], F32, space="PSUM",
                                    tag="out", name="out_p")
                den_p = ps_den.tile([128, 32], F32, space="PSUM", tag="den",
                                    name="den_p")
                for s, (c0, r, _q) in enumerate(supers):
                    G = Gs[s]
                    s_sb = erep.tile([128, 8, 16], F32, tag="s")
                    if s_pre is None:
                        pat = ps_pat.tile([128, 8, 16], F32, space="PSUM",
                                          tag="pat", name="pat")
                        for j in range(r):
                            c = c0 + j
                            nc.tensor.matmul(
                                out=pat[:, j, :],
                                lhsT=maskT_sb[:, c * 128:(c + 1) * 128],
                                rhs=at[:], start=True, stop=True)
                        nc.vector.tensor_tensor(
                            out=s_sb[:, 0:r, :], in0=pat[:, 0:r, :],
                            in1=pe_sb[:, c0:c0 + r,
                                      layer * 16:(layer + 1) * 16],
                            op=mybir.AluOpType.add)
                        nc.vector.tensor_tensor(
                            out=s_sb[:, 0:r, :], in0=s_sb[:, 0:r, :],
                            in1=G[:, 0:r, AS_OFF:AS_OFF + 16],
                            op=mybir.AluOpType.add)
                    else:
                        nc.vector.tensor_tensor(
                            out=s_sb[:, 0:r, :],
                            in0=s_pre[:, c0:c0 + r, :],
                            in1=G[:, 0:r, AS_OFF:AS_OFF + 16],
                            op=mybir.AluOpType.add)
                    e_rep = erep.tile([128, 8, 16, 2], BF16, tag="e")
                    t_rep = erep.tile([128, 8, 16, 2], BF16, tag="t")
                    for half in range(2):
                        nc.scalar.activation(
                            out=e_rep[:, 0:r, :, half], in_=s_sb[:, 0:r, :],
                            func=mybir.ActivationFunctionType.Exp, scale=1.0)
                        nc.scalar.activation(
                            out=t_rep[:, 0:r, :, half], in_=s_sb[:, 0:r, :],
                            func=mybir.ActivationFunctionType.Exp, scale=0.2)
                    nc.vector.tensor_tensor(
                        out=e_rep[:, 0:r, :, :], in0=e_rep[:, 0:r, :, :],
                        in1=t_rep[:, 0:r, :, :], op=mybir.AluOpType.max)
                    for j in range(r):
                        c = c0 + j
                        nc.vector.tensor_tensor(
                            out=G[:, j, 0:B * C].rearrange(
                                "p (x pr two) -> p x pr two", pr=32, two=2),
                            in0=G[:, j, 0:B * C].rearrange(
                                "p (x pr two) -> p x pr two", pr=32, two=2),
                            in1=e_rep[:, j, :, :].rearrange(
                                "p x (u two) -> p x u two", u=1)
                                .to_broadcast([128, B * H, 32, 2]),
                            op=mybir.AluOpType.mult)
                        mk = mask_sb[:, c * 128:(c + 1) * 128]
                        first, last = (c == 0), (c == NC_E - 1)
                        nc.tensor.matmul(out=out_p[:, 0:512], lhsT=mk,
                                         rhs=G[:, j, 0:512],
                                         start=first, stop=last)
                        nc.tensor.matmul(out=out_p[:, 512:1024], lhsT=mk,
                                         rhs=G[:, j, 512:1024],
                                         start=first, stop=last)
                        nc.tensor.matmul(
                            out=den_p[:], lhsT=mk,
                            rhs=e_rep[:, j, :, :].rearrange(
                                "p x two -> p (x two)"),
                            start=first, stop=last)
                dsb = small.tile([128, B * H], F32, tag="d")
                nc.vector.tensor_scalar_add(
                    dsb[:],
                    den_p[:].rearrange("p (x two) -> p x two", two=2)[:, :, 0],
                    1e-16)
                rec = small.tile([128, B * H], F32, tag="r")
                nc.vector.reciprocal(rec[:], dsb[:])
                xo = sb.tile([128, B * C], out_dt, tag=f"xo{layer}",
                             name=f"xo{layer}")
                nc.vector.tensor_tensor(
                    out=xo[:].rearrange("p (x d) -> p x d", d=D),
                    in0=out_p[:].rearrange("p (x d) -> p x d", d=D),
                    in1=rec[:].rearrange("p (x u) -> p x u", u=1)
                        .to_broadcast([128, B * H, D]),
                    op=mybir.AluOpType.mult)
                return xo

            x1 = edge_loop(Gs1, at1, 0, BF16)
            if debug:
                nc.sync.dma_start(out=dbg["dbg_x1"][:], in_=x1[:])
                for q in range(NQ):
                    nc.sync.dma_start(
                        out=dbg["dbg_tbl"][q * (N // NQ):(q + 1) * (N // NQ),
                                           :],
                        in_=tbls1[q][:])

            # ---- layer 2: local table rows (128 nodes) + table AllGather
            x1T_loc = sb.tile([128, 2, B * TPC], BF16)
            for b in range(B):
                for ch in range(2):
                    pt = ps_t.tile([128, 128], BF16, space="PSUM", tag="pt",
                                   name="pt")
                    nc.tensor.transpose(
                        out=pt[:],
                        in_=x1[:, b * C + ch * 128: b * C + (ch + 1) * 128],
                        identity=ident[:])
                    nc.scalar.copy(
                        out=x1T_loc[:, ch, b * 128:(b + 1) * 128], in_=pt[:])
            ag_in = dram.tile([TPC, ROW], BF16, tag="agin", name="agin")
            tbl2 = dram.tile([N, ROW], BF16, addr_space="Shared",
                             tag="tbl2", name="tbl2")
            sh2 = sb.tile([128, B, 264], BF16, name="sh2")
            for b in range(B):
                ph = ps_small.tile([128, 264], F32, space="PSUM",
                                   tag="ps", name="ph2")
                for ch in range(2):
                    nc.tensor.matmul(
                        out=ph[:],
                        lhsT=x1T_loc[:, ch, b * 128:(b + 1) * 128],
                        rhs=wa_sb["wa2"][:, ch, :],
                        start=(ch == 0), stop=(ch == 1))
                if b % 2 == 0:
                    nc.scalar.copy(out=sh2[:, b, :], in_=ph[:])
                else:
                    nc.vector.tensor_copy(out=sh2[:, b, :], in_=ph[:])
            nc.sync.dma_start(
                out=ag_in[:, 0:B * C].rearrange("n (b o) -> n b o", b=B),
                in_=sh2[:, :, 0:256])
            nc.sync.dma_start(
                out=ag_in[:, AS_OFF:AS_OFF + B * H].rearrange(
                    "n (b h) -> n b h", b=B),
                in_=sh2[:, :, 256:260])
            at2 = sb.tile([128, B * H], BF16, name="at2")
            nc.vector.tensor_copy(out=at2[:].rearrange(
                "p (b h) -> p b h", b=B), in_=sh2[:, :, 260:264])
            nc.gpsimd.collective_compute(
                "AllGather", mybir.AluOpType.bypass,
                replica_groups=[list(range(NC))],
                ins=[ag_in.opt()], outs=[tbl2.opt()])
            # precompute layer-2 score base (maskT@a_tgt + pe) under the AG
            s_pre2 = sb.tile([128, NC_E, 16], F32, name="spre2")
            for c0, r, _q in supers:
                pat = ps_pat.tile([128, 8, 16], F32, space="PSUM",
                                  tag="pat", name="pat2")
                for j in range(r):
                    c = c0 + j
                    nc.tensor.matmul(
                        out=pat[:, j, :],
                        lhsT=maskT_sb[:, c * 128:(c + 1) * 128],
                        rhs=at2[:], start=True, stop=True)
                nc.vector.tensor_tensor(
                    out=s_pre2[:, c0:c0 + r, :], in0=pat[:, 0:r, :],
                    in1=pe_sb[:, c0:c0 + r, 16:32],
                    op=mybir.AluOpType.add)
            Gs2 = issue_gathers([tbl2], isrcf_t)

            x2 = edge_loop(Gs2, at2, 1, F32, s_pre=s_pre2)
            nc.sync.dma_start(out=y_out[:], in_=x2[:])

    nc.compile()
    return nc


_CACHE: dict = {}


def _get_program(NC_P: tuple, debug: bool = False):
    key = (NC_P, debug)
    if key not in _CACHE:
        _CACHE[key] = _build(NC_P, debug)
    return _CACHE[key]


def kernel(debug=False, trace=False, **inputs):
    in_maps, NC_P = _prep(**inputs)
    nc = _get_program(NC_P, debug)
    res = run_bass_kernel_spmd(nc, in_maps, core_ids=list(range(NC)),
                               trace=trace)
    y = np.concatenate([res.results[k]["y"] for k in range(NC)], axis=0)
    out = y.reshape(N, B, C)
    if debug or trace:
        return out, res
    return out
